# revision 19
# baseline (speedup 1.0000x reference)
"""AttFlow layer kernel for Trainium2, data-parallel over batch on 8 NeuronCores.

Problem shapes (hardcoded):
  context [32, 1024, 512] f32, query [32, 256, 512] f32, w_alpha [1536] f32
  -> G [32, 1024, 2048] f32

Math per batch b (T=1024, J=256, D=512):
  S[t,j]  = c_t.w1 + q_j.w2 + sum_d c[t,d] q[j,d] w3[d]
  P       = softmax(S, axis=t)          (normalized over t, per j)
  m[t]    = max_j S[t,j];  b_t = softmax(m)
  U[t,:]  = sum_j P[t,j] q[j,:]
  h       = sum_t b_t c[t,:]
  G       = [c, U, c*U, c*h]

Kernel strategy per core (4 batches/core):
  - compute S^T [j, t] via PE matmul with d as contraction:
      lhsT = q3T[d,j] = qT*w3 + w1 (folds the c.w1 row term in),
      rhs  = cT[d,t] (PE-transposed on device)
  - exp on ACT with per-partition bias qw2[j], accum_out gives Z[j]
  - U via PE: lhsT = expS^T slices, rhs = q/Z
  - b_t path works on expS directly (max/sum of exp == exp of max trick)
  - h as [1,512] rows: lhsT = em column (K=t chunk), rhs = c; bcast via K=1 matmul
  - matmuls in float32r (1 cyc/row vs fp32's 4); G0/c*U/c*h use exact f32 c
"""

import sys
import types

import numpy as np

import concourse.bass as bass
import concourse.bass_utils as bass_utils
from concourse import mybir
from concourse.masks import make_identity
from concourse.tile import TileContext

N_CORES = 8
B, T, J, D = 32, 1024, 256, 512
BP = B // N_CORES  # batches per core
TB = T // 128  # 8 t-blocks
JB = J // 128  # 2 j-blocks
KD = D // 128  # 4 d-chunks
F32 = mybir.dt.float32
F32R = mybir.dt.float32r

_STATE = {}


def _patch_tile_drain():
    """This image's walrus rejects >1 sync-wait command on a Drain. The
    TileContext tail drain waits on every sem lane at once; split the waits
    across consecutive single-wait drains on the sync engine."""
    import bass_rust
    import concourse.tile as tile_mod
    from concourse.vector_clock import ScopedClock

    if getattr(tile_mod.TileContext, "_drain_patched", False):
        return

    def _drain_and_barrier(self, tick_clock, wait_clock):
        drain_inst = self.nc.sync.drain()
        wait_clock.add_sem_waits(
            drain_inst.ins, ScopedClock({None: tick_clock.global_clock})
        )
        waits = list(drain_inst.ins.sync_info.on_wait)
        if len(waits) > 1:
            drain_inst.ins.sync_info = bass_rust.SyncInfo(
                on_wait=waits[:1], on_update=[]
            )
            for i in range(1, len(waits)):
                extra = self.nc.sync.drain()
                extra.ins.sync_info = bass_rust.SyncInfo(
                    on_wait=waits[i : i + 1], on_update=[]
                )

        self.nc.all_engine_barrier()
        assert self.sems is not None
        popped = self.nc._tile_sem_poison_stack.pop()
        assert popped is self._sem_poison
        self.nc.clear_and_free_semaphores(list(self.sems.allocated().values()))
        self.nc.all_engine_barrier()

    tile_mod.TileContext._drain_and_barrier = _drain_and_barrier
    tile_mod.TileContext._drain_patched = True


def _split_multi_waits(nc):
    """This image's walrus allows a single sync-wait command per instruction.
    Hoist excess waits onto dedicated same-engine nops inserted immediately
    before the instruction (extra sync only — semantics preserved)."""
    import bass_rust

    for bbwrap in nc.bb_map.values():
        bb = bbwrap.bb
        insns = list(bb.instructions)
        new = []
        changed = False
        for ins in insns:
            si = ins.sync_info
            if si is not None and len(si.on_wait) > 1:
                waits = list(si.on_wait)
                for w in waits[:-1]:
                    nop = mybir.InstNoOp(
                        name=nc.get_next_instruction_name(),
                        sync_info=bass_rust.SyncInfo(on_wait=[w], on_update=[]),
                        engine=ins.engine,
                        bass_nofuse=True,
                    )
                    nc.register_instruction(nop)
                    new.append(nop)
                ins.sync_info = bass_rust.SyncInfo(
                    on_wait=[waits[-1]], on_update=list(si.on_update)
                )
                changed = True
            new.append(ins)
        if changed:
            bb.instructions = new


def build_nc(use_f32r=True):
    _patch_tile_drain()
    MMD = F32R if use_f32r else F32
    nc = bass.Bass("TRN2", target_bir_lowering=False, debug=False)

    ctx_d = nc.dram_tensor("context", [BP, T, D], F32, kind="ExternalInput")
    qry_d = nc.dram_tensor("query", [BP, J, D], F32, kind="ExternalInput")
    w_d = nc.dram_tensor("w_alpha", [3 * D], F32, kind="ExternalInput")
    out_d = nc.dram_tensor("out", [BP, T, 4 * D], F32, kind="ExternalOutput")

    ctx = ctx_d.ap()
    qry = qry_d.ap()
    w = w_d.ap()
    out = out_d.ap()

    # [b, p, n, d] view: t = n*128 + p
    ctx_r = ctx.rearrange("b (n p) d -> b p n d", p=128)
    qry_r = qry.rearrange("b (n p) d -> b p n d", p=128)

    import contextlib

    with TileContext(nc) as tc:
        with contextlib.ExitStack() as ex:
            ex.enter_context(
                nc.allow_low_precision(reason="float32r rounding is intentional")
            )
            const = ex.enter_context(tc.tile_pool(name="const", bufs=1))
            c_pool = ex.enter_context(tc.tile_pool(name="c", bufs=3))
            cr_pool = ex.enter_context(tc.tile_pool(name="cr", bufs=8))
            ct_pool = ex.enter_context(tc.tile_pool(name="ct", bufs=2))
            q_pool = ex.enter_context(tc.tile_pool(name="q", bufs=3))
            q3t_pool = ex.enter_context(tc.tile_pool(name="q3t", bufs=2))
            qs_pool = ex.enter_context(tc.tile_pool(name="qs", bufs=2))
            exps_pool = ex.enter_context(tc.tile_pool(name="exps", bufs=2))
            m1_pool = ex.enter_context(tc.tile_pool(name="m1", bufs=1))
            hb_pool = ex.enter_context(tc.tile_pool(name="hb", bufs=2))
            g_pool = ex.enter_context(tc.tile_pool(name="g", bufs=5))
            small = ex.enter_context(tc.tile_pool(name="small", bufs=4))
            tmp_pool = ex.enter_context(tc.tile_pool(name="tmp", bufs=2))

            ps_tr = ex.enter_context(tc.tile_pool(name="ps_tr", bufs=2, space="PSUM"))
            ps_s = ex.enter_context(tc.tile_pool(name="ps_s", bufs=2, space="PSUM"))
            ps_u = ex.enter_context(tc.tile_pool(name="ps_u", bufs=2, space="PSUM"))
            ps_sm = ex.enter_context(tc.tile_pool(name="ps_sm", bufs=2, space="PSUM"))

            # ---- constants ----
            ident = const.tile([128, 128], F32)
            make_identity(nc, ident[:, :])

            w1c = const.tile([128, KD], F32)  # w1 chunk k in col k
            w3c = const.tile([128, KD], F32)
            nc.sync.dma_start(w1c[:, :], w[0:D].rearrange("(c p) -> p c", p=128))
            nc.sync.dma_start(w3c[:, :], w[2 * D : 3 * D].rearrange("(c p) -> p c", p=128))

            w2s = w[D : 2 * D]
            w2b = const.tile([128, D], F32)  # w2 broadcast along partitions
            nc.sync.dma_start(
                w2b[:, :],
                bass.AP(tensor=w2s.tensor, offset=w2s.offset, ap=[[0, 128]] + list(w2s.ap)),
            )

            ones_col = const.tile([128, 1], F32)
            nc.vector.memset(ones_col[:, :], 1.0)
            ones_row = const.tile([1, 128], F32)
            nc.vector.memset(ones_row[:, :], 1.0)

            stash = {}

            def stage_a(b):
                """load + transposes + S^T matmul + exp.  PE work here only
                depends on this batch's DMAs, so it fills gaps while the
                previous batch's softmax tail runs."""
                c_sb = c_pool.tile([128, TB, D], F32, tag="c")
                nc.sync.dma_start(c_sb[:, 0:4, :], ctx_r[b, :, 0:4, :])
                nc.sync.dma_start(c_sb[:, 4:8, :], ctx_r[b, :, 4:8, :])
                q_sb = q_pool.tile([128, JB, D], F32, tag="q")
                nc.sync.dma_start(q_sb[:, :, :], qry_r[b])
                for tb in range(TB):
                    rows = slice(tb * 128, (tb + 1) * 128)
                    nc.scalar.dma_start(out[b, rows, 0:D], c_sb[:, tb, :])

                # qT -> q3T = qT*w3 + w1  (d on partitions)
                q3T = q3t_pool.tile([128, KD, J], MMD, tag="q3t")
                for k in range(KD):
                    pt = ps_tr.tile([128, J], F32, tag="ps_tr")
                    for jn in range(JB):
                        nc.tensor.transpose(
                            pt[:, jn * 128 : (jn + 1) * 128],
                            q_sb[:, jn, k * 128 : (k + 1) * 128],
                            ident[:, :],
                        )
                    nc.vector.tensor_scalar(
                        out=q3T[:, k, :],
                        in0=pt[:, :],
                        scalar1=w3c[:, k : k + 1],
                        scalar2=w1c[:, k : k + 1],
                        op0=mybir.AluOpType.mult,
                        op1=mybir.AluOpType.add,
                    )

                # cT (d on partitions); ACT copy rounds to f32r
                cT = ct_pool.tile([128, KD, T], MMD, tag="ct")
                for k in range(KD):
                    for h in range(2):
                        pct = ps_tr.tile([128, 512], F32, tag="ps_tr")
                        for n in range(4):
                            tb = h * 4 + n
                            nc.tensor.transpose(
                                pct[:, n * 128 : (n + 1) * 128],
                                c_sb[:, tb, k * 128 : (k + 1) * 128],
                                ident[:, :],
                            )
                        nc.scalar.copy(cT[:, k, h * 512 : (h + 1) * 512], pct[:, :])

                # qw2[j]
                qw2 = small.tile([128, JB], F32, tag="qw2")
                for jn in range(JB):
                    tmp = tmp_pool.tile([128, D], F32, tag="tmp")
                    nc.vector.tensor_mul(tmp[:, :], q_sb[:, jn, :], w2b[:, :])
                    nc.vector.reduce_sum(
                        out=qw2[:, jn : jn + 1], in_=tmp[:, :], axis=mybir.AxisListType.X
                    )

                # S^T matmul + exp (+row sums Z)
                expS = exps_pool.tile([128, JB, T], MMD, tag="exps")
                zpart = small.tile([128, JB, 2], F32, tag="zpart")
                for jn in range(JB):
                    for th in range(2):
                        ps = ps_s.tile([128, 512], F32, tag="ps_s")
                        for k in range(KD):
                            nc.tensor.matmul(
                                ps[:, :],
                                lhsT=q3T[:, k, jn * 128 : (jn + 1) * 128],
                                rhs=cT[:, k, th * 512 : (th + 1) * 512],
                                start=(k == 0),
                                stop=(k == KD - 1),
                            )
                        nc.scalar.activation(
                            out=expS[:, jn, th * 512 : (th + 1) * 512],
                            in_=ps[:, :],
                            func=mybir.ActivationFunctionType.Exp,
                            bias=qw2[:, jn : jn + 1],
                            accum_out=zpart[:, jn, th : th + 1],
                        )
                stash[b] = (c_sb, q_sb, expS, zpart)

            def stage_b(b):
                """softmax tail + U + G assembly + stores.

                PE order within this stage: em-transposes, ptot, U matmuls,
                h matmuls, hb — the long DVE chains (em reduce, casts) run
                while PE is busy with the U matmuls, so PE never stalls long
                enough for HAM to re-throttle."""
                c_sb, q_sb, expS, zpart = stash.pop(b)

                zinv = small.tile([128, JB], F32, tag="zinv")
                zsum = small.tile([128, JB], F32, tag="zsum")
                for jn in range(JB):
                    nc.vector.tensor_add(
                        zsum[:, jn : jn + 1], zpart[:, jn, 0:1], zpart[:, jn, 1:2]
                    )
                nc.vector.reciprocal(zinv[:, :], zsum[:, :])

                # q' = q / Z
                qs = qs_pool.tile([128, JB, D], MMD, tag="qs")
                for jn in range(JB):
                    nc.vector.tensor_scalar_mul(
                        qs[:, jn, :], q_sb[:, jn, :], zinv[:, jn : jn + 1]
                    )

                # em[t] = max_j expS^T
                m1 = m1_pool.tile([128, T], F32, tag="m1")
                nc.vector.tensor_max(m1[:, :], expS[:, 0, :], expS[:, 1, :])
                em = small.tile([128, TB], MMD, tag="em")
                for tb in range(TB):
                    pm = ps_sm.tile([128, 128], F32, tag="ps_sm")
                    nc.tensor.transpose(
                        pm[:, :], m1[:, tb * 128 : (tb + 1) * 128], ident[:, :]
                    )
                    nc.vector.reduce_max(
                        out=em[:, tb : tb + 1], in_=pm[:, :], axis=mybir.AxisListType.X
                    )

                # total = sum_t em[t]; sinv = 1/total
                esum = small.tile([128, 1], F32, tag="esum")
                nc.vector.reduce_sum(
                    out=esum[:, :], in_=em[:, :], axis=mybir.AxisListType.X
                )
                ptot = ps_sm.tile([1, 1], F32, tag="ps_sm")
                nc.tensor.matmul(
                    ptot[:, :], lhsT=esum[:, :], rhs=ones_col[:, :], start=True, stop=True
                )
                sinv = small.tile([1, 1], F32, tag="sinv")
                nc.vector.reciprocal(sinv[:, :], ptot[:, :])

                # rounded c blocks for the f32r h matmuls (emitted before the
                # U matmuls so DVE produces them while PE runs U)
                crts = []
                if use_f32r:
                    for tb in range(TB):
                        crt = cr_pool.tile([128, D], MMD, tag="cr")
                        nc.vector.tensor_copy(crt[:, :], c_sb[:, tb, :])
                        crts.append(crt)

                # U matmuls + [U, c*U] sections, stored as soon as ready
                for tb in range(TB):
                    pu = ps_u.tile([128, 512], F32, tag="ps_u")
                    for jn in range(JB):
                        nc.tensor.matmul(
                            pu[:, :],
                            lhsT=expS[:, jn, tb * 128 : (tb + 1) * 128],
                            rhs=qs[:, jn, :],
                            start=(jn == 0),
                            stop=(jn == JB - 1),
                        )
                    g2 = g_pool.tile([128, 2, D], F32, tag="g2")
                    nc.scalar.copy(g2[:, 0, :], pu[:, :])  # U
                    nc.vector.tensor_mul(g2[:, 1, :], c_sb[:, tb, :], g2[:, 0, :])  # c*U
                    rows = slice(tb * 128, (tb + 1) * 128)
                    nc.scalar.dma_start(out[b, rows, D : 3 * D], g2[:, :, :])

                # h row: h[d] = sinv * sum_t em[t] c[t,d]
                prow = ps_sm.tile([1, D], F32, tag="ps_sm")
                for tb in range(TB):
                    rhs_h = crts[tb][:, :] if use_f32r else c_sb[:, tb, :]
                    nc.tensor.matmul(
                        prow[:, :],
                        lhsT=em[:, tb : tb + 1],
                        rhs=rhs_h,
                        start=(tb == 0),
                        stop=(tb == TB - 1),
                    )
                hrow = hb_pool.tile([1, D], F32, tag="hrow")
                nc.vector.tensor_scalar_mul(hrow[:, :], prow[:, :], sinv[:, 0:1])

                phb = ps_sm.tile([128, D], F32, tag="ps_sm")
                nc.tensor.matmul(
                    phb[:, :], lhsT=ones_row[:, :], rhs=hrow[:, :], start=True, stop=True
                )
                hb = hb_pool.tile([128, D], F32, tag="hb")
                nc.scalar.copy(hb[:, :], phb[:, :])

                # c*h + remaining stores
                for tb in range(TB):
                    g3 = g_pool.tile([128, D], F32, tag="g3")
                    nc.vector.tensor_mul(g3[:, :], c_sb[:, tb, :], hb[:, :])  # c*h
                    rows = slice(tb * 128, (tb + 1) * 128)
                    nc.scalar.dma_start(out[b, rows, 3 * D : 4 * D], g3[:, :])

            # software pipeline: stage A of batch b+1 overlaps stage B of b
            stage_a(0)
            for b in range(BP):
                if b + 1 < BP:
                    stage_a(b + 1)
                stage_b(b)

    _split_multi_waits(nc)
    return nc


def _install_exec(nc):
    """Build a cached jitted SPMD executor for nc (mirrors
    bass2jax.run_bass_via_pjrt but reuses the compiled executable and
    creates output buffers on device)."""
    import jax
    import jax.numpy as jnp
    from jax.experimental.shard_map import shard_map
    from jax.sharding import Mesh, NamedSharding, PartitionSpec

    from concourse import bass2jax

    bass2jax.install_neuronx_cc_hook()

    partition_name = nc.partition_id_tensor.name if nc.partition_id_tensor else None
    in_names, out_names, out_avals = [], [], []
    for alloc in nc.m.functions[0].allocations:
        if not isinstance(alloc, mybir.MemoryLocationSet):
            continue
        name = alloc.memorylocations[0].name
        if alloc.kind == "ExternalInput":
            if name != partition_name:
                in_names.append(name)
        elif alloc.kind == "ExternalOutput":
            out_names.append(name)
            shape = tuple(alloc.tensor_shape)
            dtype = mybir.dt.np(alloc.dtype)
            out_avals.append(jax.core.ShapedArray(shape, dtype))
    n_params = len(in_names)
    n_outs = len(out_avals)
    all_in_names = list(in_names) + list(out_names)
    if partition_name is not None:
        all_in_names.append(partition_name)

    donate = tuple(range(n_params, n_params + n_outs))

    def _body(*args):
        operands = list(args)
        if partition_name is not None:
            operands.append(bass2jax.partition_id_tensor())
        outs = bass2jax._bass_exec_p.bind(
            *operands,
            out_avals=tuple(out_avals),
            in_names=tuple(all_in_names),
            out_names=tuple(out_names),
            lowering_input_output_aliases=(),
            sim_require_finite=True,
            sim_require_nnan=True,
            nc=nc,
        )
        return tuple(outs)

    devices = jax.devices()[:N_CORES]
    mesh = Mesh(np.asarray(devices), ("core",))
    in_specs = (PartitionSpec("core"),) * (n_params + n_outs)
    out_specs = (PartitionSpec("core"),) * n_outs
    sharded = jax.jit(
        shard_map(
            _body, mesh=mesh, in_specs=in_specs, out_specs=out_specs, check_rep=False
        ),
        donate_argnums=donate,
        keep_unused=True,
    )

    shard = NamedSharding(mesh, PartitionSpec("core"))
    zero_fns = [
        jax.jit(
            lambda a=a: jnp.zeros((N_CORES * a.shape[0],) + tuple(a.shape[1:]), a.dtype),
            out_shardings=shard,
        )
        for a in out_avals
    ]
    return sharded, in_names, out_names, out_avals, zero_fns


def _get_state():
    if "exec" not in _STATE:
        nc = build_nc()
        _STATE["exec"] = _install_exec(nc)
    return _STATE["exec"]


def kernel(context, query, w_alpha):
    sharded, in_names, out_names, out_avals, zero_fns = _get_state()

    context = np.ascontiguousarray(np.asarray(context, dtype=np.float32))
    query = np.ascontiguousarray(np.asarray(query, dtype=np.float32))
    w_alpha = np.ascontiguousarray(np.asarray(w_alpha, dtype=np.float32))

    # per-core shards concatenated along axis 0 (what shard_map expects)
    global_ins = {
        "context": context,  # [32,...] == concat of 8x [4,...]
        "query": query,
        "w_alpha": np.tile(w_alpha, N_CORES),  # each core gets a copy
    }
    args = [global_ins[name] for name in in_names]
    zeros = [zf() for zf in zero_fns]  # device-side, no host transfer
    out_arrs = sharded(*args, *zeros)
    out = np.asarray(out_arrs[out_names.index("out")])
    return out.reshape(B, T, 4 * D)


# revision 23
# speedup vs baseline: 1.2335x; 1.2335x over previous
"""AttFlow layer kernel for Trainium2, data-parallel over batch on 8 NeuronCores.

Problem shapes (hardcoded):
  context [32, 1024, 512] f32, query [32, 256, 512] f32, w_alpha [1536] f32
  -> G [32, 1024, 2048] f32

Math per batch b (T=1024, J=256, D=512):
  S[t,j]  = c_t.w1 + q_j.w2 + sum_d c[t,d] q[j,d] w3[d]
  P       = softmax(S, axis=t)          (normalized over t, per j)
  m[t]    = max_j S[t,j];  b_t = softmax(m)
  U[t,:]  = sum_j P[t,j] q[j,:]
  h       = sum_t b_t c[t,:]
  G       = [c, U, c*U, c*h]

Kernel strategy per core (4 batches/core):
  - compute S^T [j, t] via PE matmul with d as contraction:
      lhsT = q3T[d,j] = qT*w3 + w1 (folds the c.w1 row term in),
      rhs  = cT[d,t] (PE-transposed on device)
  - exp on ACT with per-partition bias qw2[j], accum_out gives Z[j]
  - U via PE: lhsT = expS^T slices, rhs = q/Z
  - b_t path works on expS directly (max/sum of exp == exp of max trick)
  - h as [1,512] rows: lhsT = em column (K=t chunk), rhs = c; bcast via K=1 matmul
  - matmuls in float32r (1 cyc/row vs fp32's 4); G0/c*U/c*h use exact f32 c
"""

import sys
import types

import numpy as np

import concourse.bass as bass
import concourse.bass_utils as bass_utils
from concourse import mybir
from concourse.masks import make_identity
from concourse.tile import TileContext

N_CORES = 8
B, T, J, D = 32, 1024, 256, 512
BP = B // N_CORES  # batches per core
TB = T // 128  # 8 t-blocks
JB = J // 128  # 2 j-blocks
KD = D // 128  # 4 d-chunks
F32 = mybir.dt.float32
F32R = mybir.dt.float32r

_STATE = {}


def _patch_tile_drain():
    """This image's walrus rejects >1 sync-wait command on a Drain. The
    TileContext tail drain waits on every sem lane at once; split the waits
    across consecutive single-wait drains on the sync engine."""
    import bass_rust
    import concourse.tile as tile_mod
    from concourse.vector_clock import ScopedClock

    if getattr(tile_mod.TileContext, "_drain_patched", False):
        return

    def _drain_and_barrier(self, tick_clock, wait_clock):
        drain_inst = self.nc.sync.drain()
        wait_clock.add_sem_waits(
            drain_inst.ins, ScopedClock({None: tick_clock.global_clock})
        )
        waits = list(drain_inst.ins.sync_info.on_wait)
        if len(waits) > 1:
            drain_inst.ins.sync_info = bass_rust.SyncInfo(
                on_wait=waits[:1], on_update=[]
            )
            for i in range(1, len(waits)):
                extra = self.nc.sync.drain()
                extra.ins.sync_info = bass_rust.SyncInfo(
                    on_wait=waits[i : i + 1], on_update=[]
                )

        self.nc.all_engine_barrier()
        assert self.sems is not None
        popped = self.nc._tile_sem_poison_stack.pop()
        assert popped is self._sem_poison
        self.nc.clear_and_free_semaphores(list(self.sems.allocated().values()))
        self.nc.all_engine_barrier()

    tile_mod.TileContext._drain_and_barrier = _drain_and_barrier
    tile_mod.TileContext._drain_patched = True


def _split_multi_waits(nc):
    """This image's walrus allows a single sync-wait command per instruction.
    Hoist excess waits onto dedicated same-engine nops inserted immediately
    before the instruction (extra sync only — semantics preserved)."""
    import bass_rust

    for bbwrap in nc.bb_map.values():
        bb = bbwrap.bb
        insns = list(bb.instructions)
        new = []
        changed = False
        for ins in insns:
            si = ins.sync_info
            if si is not None and len(si.on_wait) > 1:
                waits = list(si.on_wait)
                for w in waits[:-1]:
                    nop = mybir.InstNoOp(
                        name=nc.get_next_instruction_name(),
                        sync_info=bass_rust.SyncInfo(on_wait=[w], on_update=[]),
                        engine=ins.engine,
                        bass_nofuse=True,
                    )
                    nc.register_instruction(nop)
                    new.append(nop)
                ins.sync_info = bass_rust.SyncInfo(
                    on_wait=[waits[-1]], on_update=list(si.on_update)
                )
                changed = True
            new.append(ins)
        if changed:
            bb.instructions = new


def build_nc(use_f32r=True):
    _patch_tile_drain()
    MMD = F32R if use_f32r else F32
    nc = bass.Bass("TRN2", target_bir_lowering=False, debug=False)

    ctx_d = nc.dram_tensor("context", [BP, T, D], F32, kind="ExternalInput")
    qry_d = nc.dram_tensor("query", [BP, J, D], F32, kind="ExternalInput")
    w_d = nc.dram_tensor("w_alpha", [3 * D], F32, kind="ExternalInput")
    out_d = nc.dram_tensor("out", [BP, T, 4 * D], F32, kind="ExternalOutput")

    ctx = ctx_d.ap()
    qry = qry_d.ap()
    w = w_d.ap()
    out = out_d.ap()

    # [b, p, n, d] view: t = n*128 + p
    ctx_r = ctx.rearrange("b (n p) d -> b p n d", p=128)
    qry_r = qry.rearrange("b (n p) d -> b p n d", p=128)

    import contextlib

    with TileContext(nc) as tc:
        with contextlib.ExitStack() as ex:
            ex.enter_context(
                nc.allow_low_precision(reason="float32r rounding is intentional")
            )
            const = ex.enter_context(tc.tile_pool(name="const", bufs=1))
            c_pool = ex.enter_context(tc.tile_pool(name="c", bufs=4))
            cr_pool = ex.enter_context(tc.tile_pool(name="cr", bufs=4))
            ct_pool = ex.enter_context(tc.tile_pool(name="ct", bufs=2))
            q_pool = ex.enter_context(tc.tile_pool(name="q", bufs=4))
            q3t_pool = ex.enter_context(tc.tile_pool(name="q3t", bufs=2))
            qs_pool = ex.enter_context(tc.tile_pool(name="qs", bufs=2))
            exps_pool = ex.enter_context(tc.tile_pool(name="exps", bufs=2))
            m1_pool = ex.enter_context(tc.tile_pool(name="m1", bufs=1))
            hb_pool = ex.enter_context(tc.tile_pool(name="hb", bufs=2))
            g_pool = ex.enter_context(tc.tile_pool(name="g", bufs=4))
            small = ex.enter_context(tc.tile_pool(name="small", bufs=4))
            tmp_pool = ex.enter_context(tc.tile_pool(name="tmp", bufs=1))

            ps_tr = ex.enter_context(tc.tile_pool(name="ps_tr", bufs=2, space="PSUM"))
            ps_s = ex.enter_context(tc.tile_pool(name="ps_s", bufs=2, space="PSUM"))
            ps_u = ex.enter_context(tc.tile_pool(name="ps_u", bufs=2, space="PSUM"))
            ps_sm = ex.enter_context(tc.tile_pool(name="ps_sm", bufs=2, space="PSUM"))

            # ---- constants ----
            ident = const.tile([128, 128], F32)
            make_identity(nc, ident[:, :])

            w1c = const.tile([128, KD], F32)  # w1 chunk k in col k
            w3c = const.tile([128, KD], F32)
            nc.sync.dma_start(w1c[:, :], w[0:D].rearrange("(c p) -> p c", p=128))
            nc.sync.dma_start(w3c[:, :], w[2 * D : 3 * D].rearrange("(c p) -> p c", p=128))

            w2s = w[D : 2 * D]
            w2b = const.tile([128, D], F32)  # w2 broadcast along partitions
            nc.sync.dma_start(
                w2b[:, :],
                bass.AP(tensor=w2s.tensor, offset=w2s.offset, ap=[[0, 128]] + list(w2s.ap)),
            )

            ones_col = const.tile([128, 1], F32)
            nc.vector.memset(ones_col[:, :], 1.0)
            ones_row = const.tile([1, 128], F32)
            nc.vector.memset(ones_row[:, :], 1.0)

            stash = {}
            loaded = {}

            def stage_l(b):
                """input loads (SP ring, nothing queued ahead of them) plus
                the G0 = c passthrough stores (ACT ring)."""
                c_sb = c_pool.tile([128, TB, D], F32, tag="c")
                nc.sync.dma_start(c_sb[:, 0:4, :], ctx_r[b, :, 0:4, :])
                nc.sync.dma_start(c_sb[:, 4:8, :], ctx_r[b, :, 4:8, :])
                q_sb = q_pool.tile([128, JB, D], F32, tag="q")
                nc.sync.dma_start(q_sb[:, :, :], qry_r[b])
                for tb in range(TB):
                    rows = slice(tb * 128, (tb + 1) * 128)
                    nc.scalar.dma_start(out[b, rows, 0:D], c_sb[:, tb, :])
                loaded[b] = (c_sb, q_sb)

            def stage_a(b):
                """transposes + S^T matmul + exp.  PE work here only
                depends on this batch's DMAs, so it fills gaps while the
                previous batch's softmax tail runs."""
                c_sb, q_sb = loaded.pop(b)

                # qT -> q3T = qT*w3 + w1  (d on partitions)
                q3T = q3t_pool.tile([128, KD, J], MMD, tag="q3t")
                for k in range(KD):
                    pt = ps_tr.tile([128, J], F32, tag="ps_tr")
                    for jn in range(JB):
                        nc.tensor.transpose(
                            pt[:, jn * 128 : (jn + 1) * 128],
                            q_sb[:, jn, k * 128 : (k + 1) * 128],
                            ident[:, :],
                        )
                    nc.vector.tensor_scalar(
                        out=q3T[:, k, :],
                        in0=pt[:, :],
                        scalar1=w3c[:, k : k + 1],
                        scalar2=w1c[:, k : k + 1],
                        op0=mybir.AluOpType.mult,
                        op1=mybir.AluOpType.add,
                    )

                # cT (d on partitions); ACT copy rounds to f32r
                cT = ct_pool.tile([128, KD, T], MMD, tag="ct")
                for k in range(KD):
                    for h in range(2):
                        pct = ps_tr.tile([128, 512], F32, tag="ps_tr")
                        for n in range(4):
                            tb = h * 4 + n
                            nc.tensor.transpose(
                                pct[:, n * 128 : (n + 1) * 128],
                                c_sb[:, tb, k * 128 : (k + 1) * 128],
                                ident[:, :],
                            )
                        nc.scalar.copy(cT[:, k, h * 512 : (h + 1) * 512], pct[:, :])

                # qw2[j]
                qw2 = small.tile([128, JB], F32, tag="qw2")
                for jn in range(JB):
                    tmp = tmp_pool.tile([128, D], F32, tag="tmp")
                    nc.vector.tensor_mul(tmp[:, :], q_sb[:, jn, :], w2b[:, :])
                    nc.vector.reduce_sum(
                        out=qw2[:, jn : jn + 1], in_=tmp[:, :], axis=mybir.AxisListType.X
                    )

                # S^T matmul + exp (+row sums Z)
                expS = exps_pool.tile([128, JB, T], MMD, tag="exps")
                zpart = small.tile([128, JB, 2], F32, tag="zpart")
                for jn in range(JB):
                    for th in range(2):
                        ps = ps_s.tile([128, 512], F32, tag="ps_s")
                        for k in range(KD):
                            nc.tensor.matmul(
                                ps[:, :],
                                lhsT=q3T[:, k, jn * 128 : (jn + 1) * 128],
                                rhs=cT[:, k, th * 512 : (th + 1) * 512],
                                start=(k == 0),
                                stop=(k == KD - 1),
                            )
                        nc.scalar.activation(
                            out=expS[:, jn, th * 512 : (th + 1) * 512],
                            in_=ps[:, :],
                            func=mybir.ActivationFunctionType.Exp,
                            bias=qw2[:, jn : jn + 1],
                            accum_out=zpart[:, jn, th : th + 1],
                        )
                stash[b] = (c_sb, q_sb, expS, zpart)

            def stage_b(b):
                """softmax tail + U + G assembly + stores.

                PE order within this stage: em-transposes, ptot, U matmuls,
                h matmuls, hb — the long DVE chains (em reduce, casts) run
                while PE is busy with the U matmuls, so PE never stalls long
                enough for HAM to re-throttle."""
                c_sb, q_sb, expS, zpart = stash.pop(b)

                zinv = small.tile([128, JB], F32, tag="zinv")
                zsum = small.tile([128, JB], F32, tag="zsum")
                for jn in range(JB):
                    nc.vector.tensor_add(
                        zsum[:, jn : jn + 1], zpart[:, jn, 0:1], zpart[:, jn, 1:2]
                    )
                nc.vector.reciprocal(zinv[:, :], zsum[:, :])

                # q' = q / Z
                qs = qs_pool.tile([128, JB, D], MMD, tag="qs")
                for jn in range(JB):
                    nc.vector.tensor_scalar_mul(
                        qs[:, jn, :], q_sb[:, jn, :], zinv[:, jn : jn + 1]
                    )

                # em[t] = max_j expS^T
                m1 = m1_pool.tile([128, T], F32, tag="m1")
                nc.vector.tensor_max(m1[:, :], expS[:, 0, :], expS[:, 1, :])
                em = small.tile([128, TB], MMD, tag="em")
                for tb in range(TB):
                    pm = ps_sm.tile([128, 128], F32, tag="ps_sm")
                    nc.tensor.transpose(
                        pm[:, :], m1[:, tb * 128 : (tb + 1) * 128], ident[:, :]
                    )
                    nc.vector.reduce_max(
                        out=em[:, tb : tb + 1], in_=pm[:, :], axis=mybir.AxisListType.X
                    )

                # total = sum_t em[t]; sinv = 1/total
                esum = small.tile([128, 1], F32, tag="esum")
                nc.vector.reduce_sum(
                    out=esum[:, :], in_=em[:, :], axis=mybir.AxisListType.X
                )
                ptot = ps_sm.tile([1, 1], F32, tag="ps_sm")
                nc.tensor.matmul(
                    ptot[:, :], lhsT=esum[:, :], rhs=ones_col[:, :], start=True, stop=True
                )
                sinv = small.tile([1, 1], F32, tag="sinv")
                nc.vector.reciprocal(sinv[:, :], ptot[:, :])

                # rounded c blocks for the f32r h matmuls (emitted before the
                # U matmuls so DVE produces them while PE runs U)
                crts = []
                if use_f32r:
                    for tb in range(TB):
                        crt = cr_pool.tile([128, D], MMD, tag="cr")
                        nc.vector.tensor_copy(crt[:, :], c_sb[:, tb, :])
                        crts.append(crt)

                # U matmuls + [U, c*U] sections, stored as soon as ready
                for tb in range(TB):
                    pu = ps_u.tile([128, 512], F32, tag="ps_u")
                    for jn in range(JB):
                        nc.tensor.matmul(
                            pu[:, :],
                            lhsT=expS[:, jn, tb * 128 : (tb + 1) * 128],
                            rhs=qs[:, jn, :],
                            start=(jn == 0),
                            stop=(jn == JB - 1),
                        )
                    g2 = g_pool.tile([128, 2, D], F32, tag="g2")
                    nc.scalar.copy(g2[:, 0, :], pu[:, :])  # U
                    nc.vector.tensor_mul(g2[:, 1, :], c_sb[:, tb, :], g2[:, 0, :])  # c*U
                    rows = slice(tb * 128, (tb + 1) * 128)
                    nc.sync.dma_start(out[b, rows, D : 3 * D], g2[:, :, :])

                # h row: h[d] = sinv * sum_t em[t] c[t,d]
                prow = ps_sm.tile([1, D], F32, tag="ps_sm")
                for tb in range(TB):
                    rhs_h = crts[tb][:, :] if use_f32r else c_sb[:, tb, :]
                    nc.tensor.matmul(
                        prow[:, :],
                        lhsT=em[:, tb : tb + 1],
                        rhs=rhs_h,
                        start=(tb == 0),
                        stop=(tb == TB - 1),
                    )
                hrow = hb_pool.tile([1, D], F32, tag="hrow")
                nc.vector.tensor_scalar_mul(hrow[:, :], prow[:, :], sinv[:, 0:1])

                phb = ps_sm.tile([128, D], F32, tag="ps_sm")
                nc.tensor.matmul(
                    phb[:, :], lhsT=ones_row[:, :], rhs=hrow[:, :], start=True, stop=True
                )
                hb = hb_pool.tile([128, D], F32, tag="hb")
                nc.scalar.copy(hb[:, :], phb[:, :])

                # c*h + remaining stores
                for tb in range(TB):
                    g3 = g_pool.tile([128, D], F32, tag="g3")
                    nc.vector.tensor_mul(g3[:, :], c_sb[:, tb, :], hb[:, :])  # c*h
                    rows = slice(tb * 128, (tb + 1) * 128)
                    nc.scalar.dma_start(out[b, rows, 3 * D : 4 * D], g3[:, :])

            # software pipeline: loads run two batches ahead; stage A of
            # batch b+1 overlaps stage B of b
            stage_l(0)
            stage_l(1)
            stage_a(0)
            stage_l(2)
            stage_a(1)
            stage_l(3)
            stage_b(0)
            stage_a(2)
            stage_b(1)
            stage_a(3)
            stage_b(2)
            stage_b(3)

    _split_multi_waits(nc)
    return nc


def _install_exec(nc):
    """Build a cached jitted SPMD executor for nc (mirrors
    bass2jax.run_bass_via_pjrt but reuses the compiled executable and
    creates output buffers on device)."""
    import jax
    import jax.numpy as jnp
    from jax.experimental.shard_map import shard_map
    from jax.sharding import Mesh, NamedSharding, PartitionSpec

    from concourse import bass2jax

    bass2jax.install_neuronx_cc_hook()

    partition_name = nc.partition_id_tensor.name if nc.partition_id_tensor else None
    in_names, out_names, out_avals = [], [], []
    for alloc in nc.m.functions[0].allocations:
        if not isinstance(alloc, mybir.MemoryLocationSet):
            continue
        name = alloc.memorylocations[0].name
        if alloc.kind == "ExternalInput":
            if name != partition_name:
                in_names.append(name)
        elif alloc.kind == "ExternalOutput":
            out_names.append(name)
            shape = tuple(alloc.tensor_shape)
            dtype = mybir.dt.np(alloc.dtype)
            out_avals.append(jax.core.ShapedArray(shape, dtype))
    n_params = len(in_names)
    n_outs = len(out_avals)
    all_in_names = list(in_names) + list(out_names)
    if partition_name is not None:
        all_in_names.append(partition_name)

    donate = tuple(range(n_params, n_params + n_outs))

    def _body(*args):
        operands = list(args)
        if partition_name is not None:
            operands.append(bass2jax.partition_id_tensor())
        outs = bass2jax._bass_exec_p.bind(
            *operands,
            out_avals=tuple(out_avals),
            in_names=tuple(all_in_names),
            out_names=tuple(out_names),
            lowering_input_output_aliases=(),
            sim_require_finite=True,
            sim_require_nnan=True,
            nc=nc,
        )
        return tuple(outs)

    devices = jax.devices()[:N_CORES]
    mesh = Mesh(np.asarray(devices), ("core",))
    in_specs = (PartitionSpec("core"),) * (n_params + n_outs)
    out_specs = (PartitionSpec("core"),) * n_outs
    sharded = jax.jit(
        shard_map(
            _body, mesh=mesh, in_specs=in_specs, out_specs=out_specs, check_rep=False
        ),
        donate_argnums=donate,
        keep_unused=True,
    )

    shard = NamedSharding(mesh, PartitionSpec("core"))
    zero_fns = [
        jax.jit(
            lambda a=a: jnp.zeros((N_CORES * a.shape[0],) + tuple(a.shape[1:]), a.dtype),
            out_shardings=shard,
        )
        for a in out_avals
    ]
    return sharded, in_names, out_names, out_avals, zero_fns


def _get_state():
    if "exec" not in _STATE:
        nc = build_nc()
        _STATE["exec"] = _install_exec(nc)
    return _STATE["exec"]


def kernel(context, query, w_alpha):
    sharded, in_names, out_names, out_avals, zero_fns = _get_state()

    context = np.ascontiguousarray(np.asarray(context, dtype=np.float32))
    query = np.ascontiguousarray(np.asarray(query, dtype=np.float32))
    w_alpha = np.ascontiguousarray(np.asarray(w_alpha, dtype=np.float32))

    # per-core shards concatenated along axis 0 (what shard_map expects)
    global_ins = {
        "context": context,  # [32,...] == concat of 8x [4,...]
        "query": query,
        "w_alpha": np.tile(w_alpha, N_CORES),  # each core gets a copy
    }
    args = [global_ins[name] for name in in_names]
    zeros = [zf() for zf in zero_fns]  # device-side, no host transfer
    out_arrs = sharded(*args, *zeros)
    out = np.asarray(out_arrs[out_names.index("out")])
    return out.reshape(B, T, 4 * D)


# revision 24
# speedup vs baseline: 1.2478x; 1.0116x over previous
"""AttFlow layer kernel for Trainium2, data-parallel over batch on 8 NeuronCores.

Problem shapes (hardcoded):
  context [32, 1024, 512] f32, query [32, 256, 512] f32, w_alpha [1536] f32
  -> G [32, 1024, 2048] f32

Math per batch b (T=1024, J=256, D=512):
  S[t,j]  = c_t.w1 + q_j.w2 + sum_d c[t,d] q[j,d] w3[d]
  P       = softmax(S, axis=t)          (normalized over t, per j)
  m[t]    = max_j S[t,j];  b_t = softmax(m)
  U[t,:]  = sum_j P[t,j] q[j,:]
  h       = sum_t b_t c[t,:]
  G       = [c, U, c*U, c*h]

Kernel strategy per core (4 batches/core):
  - compute S^T [j, t] via PE matmul with d as contraction:
      lhsT = q3T[d,j] = qT*w3 + w1 (folds the c.w1 row term in),
      rhs  = cT[d,t] (PE-transposed on device)
  - exp on ACT with per-partition bias qw2[j], accum_out gives Z[j]
  - U via PE: lhsT = expS^T slices, rhs = q/Z
  - b_t path works on expS directly (max/sum of exp == exp of max trick)
  - h as [1,512] rows: lhsT = em column (K=t chunk), rhs = c; bcast via K=1 matmul
  - matmuls in float32r (1 cyc/row vs fp32's 4); G0/c*U/c*h use exact f32 c
"""

import sys
import types

import numpy as np

import concourse.bass as bass
import concourse.bass_utils as bass_utils
from concourse import mybir
from concourse.masks import make_identity
from concourse.tile import TileContext

N_CORES = 8
B, T, J, D = 32, 1024, 256, 512
BP = B // N_CORES  # batches per core
TB = T // 128  # 8 t-blocks
JB = J // 128  # 2 j-blocks
KD = D // 128  # 4 d-chunks
F32 = mybir.dt.float32
F32R = mybir.dt.float32r

_STATE = {}


def _patch_tile_drain():
    """This image's walrus rejects >1 sync-wait command on a Drain. The
    TileContext tail drain waits on every sem lane at once; split the waits
    across consecutive single-wait drains on the sync engine."""
    import bass_rust
    import concourse.tile as tile_mod
    from concourse.vector_clock import ScopedClock

    if getattr(tile_mod.TileContext, "_drain_patched", False):
        return

    def _drain_and_barrier(self, tick_clock, wait_clock):
        drain_inst = self.nc.sync.drain()
        wait_clock.add_sem_waits(
            drain_inst.ins, ScopedClock({None: tick_clock.global_clock})
        )
        waits = list(drain_inst.ins.sync_info.on_wait)
        if len(waits) > 1:
            drain_inst.ins.sync_info = bass_rust.SyncInfo(
                on_wait=waits[:1], on_update=[]
            )
            for i in range(1, len(waits)):
                extra = self.nc.sync.drain()
                extra.ins.sync_info = bass_rust.SyncInfo(
                    on_wait=waits[i : i + 1], on_update=[]
                )

        self.nc.all_engine_barrier()
        assert self.sems is not None
        popped = self.nc._tile_sem_poison_stack.pop()
        assert popped is self._sem_poison
        self.nc.clear_and_free_semaphores(list(self.sems.allocated().values()))
        self.nc.all_engine_barrier()

    tile_mod.TileContext._drain_and_barrier = _drain_and_barrier
    tile_mod.TileContext._drain_patched = True


def _split_multi_waits(nc):
    """This image's walrus allows a single sync-wait command per instruction.
    Hoist excess waits onto dedicated same-engine nops inserted immediately
    before the instruction (extra sync only — semantics preserved)."""
    import bass_rust

    for bbwrap in nc.bb_map.values():
        bb = bbwrap.bb
        insns = list(bb.instructions)
        new = []
        changed = False
        for ins in insns:
            si = ins.sync_info
            if si is not None and len(si.on_wait) > 1:
                waits = list(si.on_wait)
                for w in waits[:-1]:
                    nop = mybir.InstNoOp(
                        name=nc.get_next_instruction_name(),
                        sync_info=bass_rust.SyncInfo(on_wait=[w], on_update=[]),
                        engine=ins.engine,
                        bass_nofuse=True,
                    )
                    nc.register_instruction(nop)
                    new.append(nop)
                ins.sync_info = bass_rust.SyncInfo(
                    on_wait=[waits[-1]], on_update=list(si.on_update)
                )
                changed = True
            new.append(ins)
        if changed:
            bb.instructions = new


def build_nc(use_f32r=True):
    _patch_tile_drain()
    MMD = F32R if use_f32r else F32
    nc = bass.Bass("TRN2", target_bir_lowering=False, debug=False)

    ctx_d = nc.dram_tensor("context", [BP, T, D], F32, kind="ExternalInput")
    qry_d = nc.dram_tensor("query", [BP, J, D], F32, kind="ExternalInput")
    w_d = nc.dram_tensor("w_alpha", [3 * D], F32, kind="ExternalInput")
    out_d = nc.dram_tensor("out", [BP, T, 4 * D], F32, kind="ExternalOutput")

    ctx = ctx_d.ap()
    qry = qry_d.ap()
    w = w_d.ap()
    out = out_d.ap()

    # [b, p, n, d] view: t = n*128 + p
    ctx_r = ctx.rearrange("b (n p) d -> b p n d", p=128)
    qry_r = qry.rearrange("b (n p) d -> b p n d", p=128)

    import contextlib

    with TileContext(nc) as tc:
        with contextlib.ExitStack() as ex:
            ex.enter_context(
                nc.allow_low_precision(reason="float32r rounding is intentional")
            )
            const = ex.enter_context(tc.tile_pool(name="const", bufs=1))
            c_pool = ex.enter_context(tc.tile_pool(name="c", bufs=4))
            cr_pool = ex.enter_context(tc.tile_pool(name="cr", bufs=4))
            ct_pool = ex.enter_context(tc.tile_pool(name="ct", bufs=2))
            q_pool = ex.enter_context(tc.tile_pool(name="q", bufs=4))
            q3t_pool = ex.enter_context(tc.tile_pool(name="q3t", bufs=2))
            qs_pool = ex.enter_context(tc.tile_pool(name="qs", bufs=2))
            exps_pool = ex.enter_context(tc.tile_pool(name="exps", bufs=2))
            m1_pool = ex.enter_context(tc.tile_pool(name="m1", bufs=1))
            hb_pool = ex.enter_context(tc.tile_pool(name="hb", bufs=2))
            g_pool = ex.enter_context(tc.tile_pool(name="g", bufs=4))
            small = ex.enter_context(tc.tile_pool(name="small", bufs=4))
            tmp_pool = ex.enter_context(tc.tile_pool(name="tmp", bufs=1))

            ps_tr = ex.enter_context(tc.tile_pool(name="ps_tr", bufs=2, space="PSUM"))
            ps_s = ex.enter_context(tc.tile_pool(name="ps_s", bufs=2, space="PSUM"))
            ps_u = ex.enter_context(tc.tile_pool(name="ps_u", bufs=2, space="PSUM"))
            ps_sm = ex.enter_context(tc.tile_pool(name="ps_sm", bufs=2, space="PSUM"))

            # ---- constants ----
            ident = const.tile([128, 128], F32)
            make_identity(nc, ident[:, :])

            w1c = const.tile([128, KD], F32)  # w1 chunk k in col k
            w3c = const.tile([128, KD], F32)
            nc.sync.dma_start(w1c[:, :], w[0:D].rearrange("(c p) -> p c", p=128))
            nc.sync.dma_start(w3c[:, :], w[2 * D : 3 * D].rearrange("(c p) -> p c", p=128))

            w2s = w[D : 2 * D]
            w2b = const.tile([128, D], F32)  # w2 broadcast along partitions
            nc.sync.dma_start(
                w2b[:, :],
                bass.AP(tensor=w2s.tensor, offset=w2s.offset, ap=[[0, 128]] + list(w2s.ap)),
            )

            ones_col = const.tile([128, 1], F32)
            nc.vector.memset(ones_col[:, :], 1.0)
            ones_row = const.tile([1, 128], F32)
            nc.vector.memset(ones_row[:, :], 1.0)

            stash = {}
            loaded = {}

            def stage_l(b):
                """input loads (SP ring, nothing queued ahead of them) plus
                the G0 = c passthrough stores (ACT ring)."""
                c_sb = c_pool.tile([128, TB, D], F32, tag="c")
                nc.sync.dma_start(c_sb[:, 0:4, :], ctx_r[b, :, 0:4, :])
                nc.sync.dma_start(c_sb[:, 4:8, :], ctx_r[b, :, 4:8, :])
                q_sb = q_pool.tile([128, JB, D], F32, tag="q")
                nc.sync.dma_start(q_sb[:, :, :], qry_r[b])
                loaded[b] = (c_sb, q_sb)

            def stage_a(b):
                """transposes + S^T matmul + exp.  PE work here only
                depends on this batch's DMAs, so it fills gaps while the
                previous batch's softmax tail runs."""
                c_sb, q_sb = loaded.pop(b)

                # qT -> q3T = qT*w3 + w1  (d on partitions)
                q3T = q3t_pool.tile([128, KD, J], MMD, tag="q3t")
                for k in range(KD):
                    pt = ps_tr.tile([128, J], F32, tag="ps_tr")
                    for jn in range(JB):
                        nc.tensor.transpose(
                            pt[:, jn * 128 : (jn + 1) * 128],
                            q_sb[:, jn, k * 128 : (k + 1) * 128],
                            ident[:, :],
                        )
                    nc.vector.tensor_scalar(
                        out=q3T[:, k, :],
                        in0=pt[:, :],
                        scalar1=w3c[:, k : k + 1],
                        scalar2=w1c[:, k : k + 1],
                        op0=mybir.AluOpType.mult,
                        op1=mybir.AluOpType.add,
                    )

                # cT (d on partitions); ACT copy rounds to f32r
                cT = ct_pool.tile([128, KD, T], MMD, tag="ct")
                for k in range(KD):
                    for h in range(2):
                        pct = ps_tr.tile([128, 512], F32, tag="ps_tr")
                        for n in range(4):
                            tb = h * 4 + n
                            nc.tensor.transpose(
                                pct[:, n * 128 : (n + 1) * 128],
                                c_sb[:, tb, k * 128 : (k + 1) * 128],
                                ident[:, :],
                            )
                        nc.scalar.copy(cT[:, k, h * 512 : (h + 1) * 512], pct[:, :])

                # qw2[j]
                qw2 = small.tile([128, JB], F32, tag="qw2")
                for jn in range(JB):
                    tmp = tmp_pool.tile([128, D], F32, tag="tmp")
                    nc.vector.tensor_mul(tmp[:, :], q_sb[:, jn, :], w2b[:, :])
                    nc.vector.reduce_sum(
                        out=qw2[:, jn : jn + 1], in_=tmp[:, :], axis=mybir.AxisListType.X
                    )

                # S^T matmul + exp (+row sums Z)
                expS = exps_pool.tile([128, JB, T], MMD, tag="exps")
                zpart = small.tile([128, JB, 2], F32, tag="zpart")
                for jn in range(JB):
                    for th in range(2):
                        ps = ps_s.tile([128, 512], F32, tag="ps_s")
                        for k in range(KD):
                            nc.tensor.matmul(
                                ps[:, :],
                                lhsT=q3T[:, k, jn * 128 : (jn + 1) * 128],
                                rhs=cT[:, k, th * 512 : (th + 1) * 512],
                                start=(k == 0),
                                stop=(k == KD - 1),
                            )
                        nc.scalar.activation(
                            out=expS[:, jn, th * 512 : (th + 1) * 512],
                            in_=ps[:, :],
                            func=mybir.ActivationFunctionType.Exp,
                            bias=qw2[:, jn : jn + 1],
                            accum_out=zpart[:, jn, th : th + 1],
                        )
                stash[b] = (c_sb, q_sb, expS, zpart)

            def stage_b(b):
                """softmax tail + U + G assembly + stores.

                PE order within this stage: em-transposes, ptot, U matmuls,
                h matmuls, hb — the long DVE chains (em reduce, casts) run
                while PE is busy with the U matmuls, so PE never stalls long
                enough for HAM to re-throttle."""
                c_sb, q_sb, expS, zpart = stash.pop(b)

                # G0 = c passthrough stores (ACT ring); emitted here so the
                # pipeline-fill loads get the DMA queues to themselves
                for tb in range(TB):
                    rows = slice(tb * 128, (tb + 1) * 128)
                    nc.scalar.dma_start(out[b, rows, 0:D], c_sb[:, tb, :])

                zinv = small.tile([128, JB], F32, tag="zinv")
                zsum = small.tile([128, JB], F32, tag="zsum")
                for jn in range(JB):
                    nc.vector.tensor_add(
                        zsum[:, jn : jn + 1], zpart[:, jn, 0:1], zpart[:, jn, 1:2]
                    )
                nc.vector.reciprocal(zinv[:, :], zsum[:, :])

                # q' = q / Z
                qs = qs_pool.tile([128, JB, D], MMD, tag="qs")
                for jn in range(JB):
                    nc.vector.tensor_scalar_mul(
                        qs[:, jn, :], q_sb[:, jn, :], zinv[:, jn : jn + 1]
                    )

                # em[t] = max_j expS^T
                m1 = m1_pool.tile([128, T], F32, tag="m1")
                nc.vector.tensor_max(m1[:, :], expS[:, 0, :], expS[:, 1, :])
                em = small.tile([128, TB], MMD, tag="em")
                for tb in range(TB):
                    pm = ps_sm.tile([128, 128], F32, tag="ps_sm")
                    nc.tensor.transpose(
                        pm[:, :], m1[:, tb * 128 : (tb + 1) * 128], ident[:, :]
                    )
                    nc.vector.reduce_max(
                        out=em[:, tb : tb + 1], in_=pm[:, :], axis=mybir.AxisListType.X
                    )

                # total = sum_t em[t]; sinv = 1/total
                esum = small.tile([128, 1], F32, tag="esum")
                nc.vector.reduce_sum(
                    out=esum[:, :], in_=em[:, :], axis=mybir.AxisListType.X
                )
                ptot = ps_sm.tile([1, 1], F32, tag="ps_sm")
                nc.tensor.matmul(
                    ptot[:, :], lhsT=esum[:, :], rhs=ones_col[:, :], start=True, stop=True
                )
                sinv = small.tile([1, 1], F32, tag="sinv")
                nc.vector.reciprocal(sinv[:, :], ptot[:, :])

                # rounded c blocks for the f32r h matmuls (emitted before the
                # U matmuls so DVE produces them while PE runs U)
                crts = []
                if use_f32r:
                    for tb in range(TB):
                        crt = cr_pool.tile([128, D], MMD, tag="cr")
                        nc.vector.tensor_copy(crt[:, :], c_sb[:, tb, :])
                        crts.append(crt)

                # U matmuls + [U, c*U] sections, stored as soon as ready
                for tb in range(TB):
                    pu = ps_u.tile([128, 512], F32, tag="ps_u")
                    for jn in range(JB):
                        nc.tensor.matmul(
                            pu[:, :],
                            lhsT=expS[:, jn, tb * 128 : (tb + 1) * 128],
                            rhs=qs[:, jn, :],
                            start=(jn == 0),
                            stop=(jn == JB - 1),
                        )
                    g2 = g_pool.tile([128, 2, D], F32, tag="g2")
                    nc.scalar.copy(g2[:, 0, :], pu[:, :])  # U
                    nc.vector.tensor_mul(g2[:, 1, :], c_sb[:, tb, :], g2[:, 0, :])  # c*U
                    rows = slice(tb * 128, (tb + 1) * 128)
                    nc.sync.dma_start(out[b, rows, D : 3 * D], g2[:, :, :])

                # h row: h[d] = sinv * sum_t em[t] c[t,d]
                prow = ps_sm.tile([1, D], F32, tag="ps_sm")
                for tb in range(TB):
                    rhs_h = crts[tb][:, :] if use_f32r else c_sb[:, tb, :]
                    nc.tensor.matmul(
                        prow[:, :],
                        lhsT=em[:, tb : tb + 1],
                        rhs=rhs_h,
                        start=(tb == 0),
                        stop=(tb == TB - 1),
                    )
                hrow = hb_pool.tile([1, D], F32, tag="hrow")
                nc.vector.tensor_scalar_mul(hrow[:, :], prow[:, :], sinv[:, 0:1])

                phb = ps_sm.tile([128, D], F32, tag="ps_sm")
                nc.tensor.matmul(
                    phb[:, :], lhsT=ones_row[:, :], rhs=hrow[:, :], start=True, stop=True
                )
                hb = hb_pool.tile([128, D], F32, tag="hb")
                nc.scalar.copy(hb[:, :], phb[:, :])

                # c*h + remaining stores
                for tb in range(TB):
                    g3 = g_pool.tile([128, D], F32, tag="g3")
                    nc.vector.tensor_mul(g3[:, :], c_sb[:, tb, :], hb[:, :])  # c*h
                    rows = slice(tb * 128, (tb + 1) * 128)
                    nc.scalar.dma_start(out[b, rows, 3 * D : 4 * D], g3[:, :])

            # software pipeline: loads run two batches ahead; stage A of
            # batch b+1 overlaps stage B of b
            stage_l(0)
            stage_l(1)
            stage_a(0)
            stage_l(2)
            stage_a(1)
            stage_l(3)
            stage_b(0)
            stage_a(2)
            stage_b(1)
            stage_a(3)
            stage_b(2)
            stage_b(3)

    _split_multi_waits(nc)
    return nc


def _install_exec(nc):
    """Build a cached jitted SPMD executor for nc (mirrors
    bass2jax.run_bass_via_pjrt but reuses the compiled executable and
    creates output buffers on device)."""
    import jax
    import jax.numpy as jnp
    from jax.experimental.shard_map import shard_map
    from jax.sharding import Mesh, NamedSharding, PartitionSpec

    from concourse import bass2jax

    bass2jax.install_neuronx_cc_hook()

    partition_name = nc.partition_id_tensor.name if nc.partition_id_tensor else None
    in_names, out_names, out_avals = [], [], []
    for alloc in nc.m.functions[0].allocations:
        if not isinstance(alloc, mybir.MemoryLocationSet):
            continue
        name = alloc.memorylocations[0].name
        if alloc.kind == "ExternalInput":
            if name != partition_name:
                in_names.append(name)
        elif alloc.kind == "ExternalOutput":
            out_names.append(name)
            shape = tuple(alloc.tensor_shape)
            dtype = mybir.dt.np(alloc.dtype)
            out_avals.append(jax.core.ShapedArray(shape, dtype))
    n_params = len(in_names)
    n_outs = len(out_avals)
    all_in_names = list(in_names) + list(out_names)
    if partition_name is not None:
        all_in_names.append(partition_name)

    donate = tuple(range(n_params, n_params + n_outs))

    def _body(*args):
        operands = list(args)
        if partition_name is not None:
            operands.append(bass2jax.partition_id_tensor())
        outs = bass2jax._bass_exec_p.bind(
            *operands,
            out_avals=tuple(out_avals),
            in_names=tuple(all_in_names),
            out_names=tuple(out_names),
            lowering_input_output_aliases=(),
            sim_require_finite=True,
            sim_require_nnan=True,
            nc=nc,
        )
        return tuple(outs)

    devices = jax.devices()[:N_CORES]
    mesh = Mesh(np.asarray(devices), ("core",))
    in_specs = (PartitionSpec("core"),) * (n_params + n_outs)
    out_specs = (PartitionSpec("core"),) * n_outs
    sharded = jax.jit(
        shard_map(
            _body, mesh=mesh, in_specs=in_specs, out_specs=out_specs, check_rep=False
        ),
        donate_argnums=donate,
        keep_unused=True,
    )

    shard = NamedSharding(mesh, PartitionSpec("core"))
    zero_fns = [
        jax.jit(
            lambda a=a: jnp.zeros((N_CORES * a.shape[0],) + tuple(a.shape[1:]), a.dtype),
            out_shardings=shard,
        )
        for a in out_avals
    ]
    return sharded, in_names, out_names, out_avals, zero_fns


def _get_state():
    if "exec" not in _STATE:
        nc = build_nc()
        _STATE["exec"] = _install_exec(nc)
    return _STATE["exec"]


def kernel(context, query, w_alpha):
    sharded, in_names, out_names, out_avals, zero_fns = _get_state()

    context = np.ascontiguousarray(np.asarray(context, dtype=np.float32))
    query = np.ascontiguousarray(np.asarray(query, dtype=np.float32))
    w_alpha = np.ascontiguousarray(np.asarray(w_alpha, dtype=np.float32))

    # per-core shards concatenated along axis 0 (what shard_map expects)
    global_ins = {
        "context": context,  # [32,...] == concat of 8x [4,...]
        "query": query,
        "w_alpha": np.tile(w_alpha, N_CORES),  # each core gets a copy
    }
    args = [global_ins[name] for name in in_names]
    zeros = [zf() for zf in zero_fns]  # device-side, no host transfer
    out_arrs = sharded(*args, *zeros)
    out = np.asarray(out_arrs[out_names.index("out")])
    return out.reshape(B, T, 4 * D)
